# revision 13
# baseline (speedup 1.0000x reference)
"""BlockSparseLocallyConnected forward on 8 Trainium2 NeuronCores.

Data-parallel over batch: 8 images per core, weights replicated.

out[b, nr, nc] = sum_{dr,dc} xpad[b, 16*nr+dr, 16*nc+dc] * w[(nr,nc), dr*32+dc] + bias

Decomposition: dr = 16*h + u, dc = 16*i + v (h,i in {0,1}; u,v in [0,16)),
nr = 8*g + j (g in [0,4), j in [0,8)).  Patch row = 128*g + 16*(j+h) + u.
SBUF holds, per (b,g), a [128, 2, FULL] tile whose h=0 half comes from HBM
(rows 128g..128g+127) and whose h=1 half (rows 16..143 of the group span)
is built ON-CHIP by two SBUF->SBUF DMA copies on the ACT ring (partitions
16..127 of the same tile's h0 half, plus partitions 0..15 of the next
group's h0 half / an HBM tail tile for g=3).  This halves HBM x traffic.

Per (b, g): DVE tensor_mul (bf16) -> product [128, (h,nc,i,v)=2048].
PE matmul with 0/1 selector lhsT L_g[16j+u, 8g+j] reduces u over partitions
and accumulates (g, h) into PSUM [128, (nc,i,v)], 4 batches per PSUM tile
(col-tile offsets 0/32/64/96).  Four of the 32 products are offloaded to
the otherwise-idle GpSimd engine (their selector matmuls are deferred to
just before the owning batch's stop-quad; PSUM accumulation commutes).
DVE tensor_reduce(axis=X) folds (i,v), then bias add.  All layout
shuffles/casts are host-side numpy so every DMA is a contiguous 1:1 copy.
"""

import os
import sys

sys.path.insert(0, "/opt/trn_rl_repo")

import numpy as np
import ml_dtypes

# ---- problem constants (hardcoded; kernel.py must be self-contained) ----
B = 64            # batch
H = W = 512
PH = PW = 8
FULL = 528        # padded H/W
NKH = NKW = 32    # window grid
NCORES = 8
BL = B // NCORES  # batches per core = 8
G = 4             # window-row groups of 8 (nr = 8g + j)
WAVES = BL // 4   # psum waves per core = 2

BF16 = ml_dtypes.bfloat16

_CACHE = {}

TRACE = False          # test.py sets True to get exec_time_ns
LAST_RESULTS = None    # BassKernelResults of last run (for test.py)

# products computed on GpSimd instead of DVE: (g, b) with late PE deadlines
OFFLOAD = ((1, 6), (1, 7), (2, 6), (2, 7))


def _build_program():
    import concourse.bass as bass
    import concourse.bacc as bacc
    import concourse.tile as tile
    from concourse import mybir

    dt_c = mybir.dt.bfloat16
    f32 = mybir.dt.float32

    nc = bacc.Bacc(
        "TRN2", target_bir_lowering=False, debug=False, num_devices=NCORES
    )
    xs = nc.dram_tensor("xs", [BL, G, 128, FULL], dt_c, kind="ExternalInput")
    xt = nc.dram_tensor("xt", [BL, 16, FULL], dt_c, kind="ExternalInput")
    wp = nc.dram_tensor("wp", [128, G, 2, 2, 32, 16], dt_c, kind="ExternalInput")
    lm = nc.dram_tensor("lm", [128, G, 32], dt_c, kind="ExternalInput")
    bp = nc.dram_tensor("bp", [128, 32], f32, kind="ExternalInput")
    out_d = nc.dram_tensor("out", [WAVES, 128, 32], f32, kind="ExternalOutput")

    with tile.TileContext(nc) as tc:
        with (
            tc.tile_pool(name="xpool", bufs=BL * G) as xpool,
            tc.tile_pool(name="tpool", bufs=BL) as tpool,
            tc.tile_pool(name="cst", bufs=1) as cst,
            tc.tile_pool(name="ppool", bufs=6) as ppool,
            tc.tile_pool(name="gpool", bufs=4) as gpool,
            tc.tile_pool(name="psum", bufs=2, space="PSUM") as psum,
            tc.tile_pool(name="opool", bufs=2) as opool,
        ):
            l_sb = cst.tile([128, G, 32], dt_c)
            w_all = cst.tile([128, G, 2, 2, 32, 16], dt_c)
            b_sb = cst.tile([128, 32], f32)
            x_sb = [[None] * G for _ in range(BL)]
            for b in range(BL):
                for g in range(G):
                    x_sb[b][g] = xpool.tile(
                        [128, 2, FULL], dt_c, tag="xb", name=f"xb_{b}_{g}"
                    )
            xt_sb = [
                tpool.tile([16, FULL], dt_c, tag="xt", name=f"xt_{b}")
                for b in range(BL)
            ]

            # ---- HBM stream (SP ring), in consumption order.  g0 and g1
            # tiles interleave per-b because (g0,b)'s h1 half needs
            # (b,g1)'s h0 rows.
            for b in range(BL):
                nc.sync.dma_start(out=x_sb[b][0][:, 0], in_=xs[b, 0])
                nc.sync.dma_start(out=x_sb[b][1][:, 0], in_=xs[b, 1])
                if b == 0:
                    nc.sync.dma_start(out=w_all[:, 0, 0], in_=wp[:, 0, 0])
                    nc.sync.dma_start(out=w_all[:, 0, 1], in_=wp[:, 0, 1])
                    nc.sync.dma_start(out=l_sb[:], in_=lm[:])
                if b == 3:
                    nc.sync.dma_start(out=w_all[:, 1], in_=wp[:, 1])
            for b in range(BL):
                nc.sync.dma_start(out=xt_sb[b][:], in_=xt[b])
            nc.sync.dma_start(out=w_all[:, 2], in_=wp[:, 2])
            for b in range(BL):
                nc.sync.dma_start(out=x_sb[b][2][:, 0], in_=xs[b, 2])
            nc.sync.dma_start(out=w_all[:, 3], in_=wp[:, 3])
            for b in range(BL):
                nc.sync.dma_start(out=x_sb[b][3][:, 0], in_=xs[b, 3])
            nc.sync.dma_start(out=b_sb[:], in_=bp[:])

            # ---- on-chip h1-half construction (ACT ring, SBUF->SBUF).
            # h1 rows 16..143 of the group span = own h0 partitions 16..127
            # + next tile's h0 partitions 0..15 (HBM tail tile for g=3).
            for g in range(G):
                for b in range(BL):
                    nc.scalar.dma_start(
                        out=x_sb[b][g][0:112, 1], in_=x_sb[b][g][16:128, 0]
                    )
                    if g < G - 1:
                        nc.scalar.dma_start(
                            out=x_sb[b][g][112:128, 1],
                            in_=x_sb[b][g + 1][0:16, 0],
                        )
                    else:
                        nc.scalar.dma_start(
                            out=x_sb[b][g][112:128, 1], in_=xt_sb[b][0:16]
                        )

            # ---- warmup: HAM downclocks the whole core (PE and the DMA
            # fabric) until ~3.4us of sustained PE activity, so start
            # back-to-back matmuls as early as possible.
            warm = cst.tile([128, 512], dt_c)
            nc.gpsimd.memset(warm[:], 0.0)
            wpsum = psum.tile([128, 512], f32, tag="warm")
            for _ in range(12):
                nc.tensor.matmul(wpsum[:], warm[:, 0:128], warm[:],
                                 start=True, stop=True)
            # GpSimd extended-tensor_tensor ucode pre-warm (~3us IRAM load
            # on first multi-dim TT op).
            gwarm = cst.tile([8, 32, 16], f32)
            nc.gpsimd.memset(gwarm[:], 0.0)
            nc.gpsimd.tensor_add(
                gwarm[:, :, 0:8], gwarm[:, :, 8:16], gwarm[:, :, 8:16]
            )
            nc.gpsimd.tensor_add(gwarm[:, 0, :], gwarm[:, 1, :], gwarm[:, 2, :])

            ps_tiles = [
                psum.tile([128, 32, 16], f32, tag="acc", name=f"acc{w}")
                for w in range(WAVES)
            ]

            def make_views(b, g, sh, n_s):
                base = x_sb[b][g][:]
                s_dim = [] if n_s == 1 else [[FULL, 2]]
                xview = bass.AP(
                    tensor=base.tensor,
                    offset=base.offset + sh * FULL,
                    ap=[
                        list(base.ap[0]),   # partition
                        *s_dim,             # shift (absent if split)
                        [16, 2],            # i (col offset 16i)
                        [1, 512],           # nc*16+v contiguous
                    ],
                )
                ws_dim = [] if n_s == 1 else [[1024, 2]]
                wview = bass.AP(
                    tensor=w_all.tensor,
                    offset=w_all.offset + g * 2048 + sh * 1024,
                    ap=[
                        list(w_all.ap[0]),  # partition
                        *ws_dim,            # h (=shift)
                        [512, 2],           # i
                        [1, 512],           # nc*16+v
                    ],
                )
                return xview, wview

            def pview_of(prod, sh, n_s):
                ps_dim = [] if n_s == 1 else [[1024, 2]]
                return bass.AP(
                    tensor=prod.tensor,
                    offset=prod.offset + sh * 1024,
                    ap=[list(prod.ap[0]), *ps_dim, [512, 2], [1, 512]],
                )

            def emit_quad(b, g, prod):
                wv, c = divmod(b, 4)
                for s in range(2):
                    for i in range(2):
                        nc.tensor.matmul(
                            ps_tiles[wv][32 * c : 32 * c + 32, :, :],
                            l_sb[:, g, :],
                            prod[:, s, i],
                            start=(g == 0 and i == 0 and s == 0),
                            stop=(g == G - 1 and i == 1 and s == 1),
                            tile_position=(0, 32 * c),
                        )

            deferred = {}  # b -> list of (g, prod) quads to emit late
            for g in range(G):
                for b in range(BL):
                    if (g, b) in OFFLOAD:
                        prod = gpool.tile([128, 2, 2, 512], dt_c, tag="gprod")
                        xv, wv_ = make_views(b, g, 0, 2)
                        nc.gpsimd.tensor_mul(pview_of(prod, 0, 2), xv, wv_)
                        deferred.setdefault(b, []).append((g, prod))
                        continue
                    prod = ppool.tile([128, 2, 2, 512], dt_c, tag="prod")
                    # g0 products are s-split: the h0 half only needs the
                    # HBM tile, so DVE starts before the h1 copies land.
                    halves = (0, 1) if g == 0 else (0,)
                    n_s = 1 if g == 0 else 2
                    for sh in halves:
                        xv, wv_ = make_views(b, g, sh, n_s)
                        nc.vector.tensor_mul(pview_of(prod, sh, n_s), xv, wv_)
                    if g == G - 1 and b in deferred:
                        for g_, prod_ in deferred.pop(b):
                            emit_quad(b, g_, prod_)
                    emit_quad(b, g, prod)

            def emit_wave_out(wv):
                # Wave 0 retires on ACT+GpSimd (PSUM drain + pairwise v-fold
                # + bias) while DVE still streams; wave 1 on DVE at the end.
                if wv == 0:
                    cp = opool.tile([128, 32, 16], f32, tag="cp")
                    nc.scalar.copy(cp[:], ps_tiles[0][:])
                    prev = cp
                    for w_ in (8, 4, 2):
                        nxt = opool.tile([128, 32, w_], f32, tag=f"t{w_}")
                        nc.gpsimd.tensor_add(
                            nxt[:], prev[:, :, 0:w_], prev[:, :, w_ : 2 * w_]
                        )
                        prev = nxt
                    tmp = opool.tile([128, 32], f32, tag="tmp0")
                    nc.gpsimd.tensor_add(tmp[:], prev[:, :, 0], prev[:, :, 1])
                    ow = opool.tile([128, 32], f32, tag="ow0")
                    nc.gpsimd.tensor_add(ow[:], tmp[:], b_sb[:])
                else:
                    tmp = opool.tile([128, 32], f32, tag="tmp")
                    nc.vector.tensor_reduce(
                        tmp[:], ps_tiles[wv][:],
                        axis=mybir.AxisListType.X, op=mybir.AluOpType.add,
                    )
                    ow = opool.tile([128, 32], f32, tag="ow")
                    nc.vector.tensor_add(ow[:], tmp[:], b_sb[:])
                nc.scalar.dma_start(out=out_d[wv], in_=ow[:])

            emit_wave_out(0)
            emit_wave_out(1)
    nc.compile()
    return nc


def _prep_inputs(x, weight, bias):
    """Host-side packing: pad, (j,u)-major weight shuffle, bf16 cast.
    Returns per-core in_maps."""
    x = np.asarray(x, dtype=np.float32)
    weight = np.asarray(weight, dtype=np.float32)
    bias = np.asarray(bias, dtype=np.float32)

    xp = np.zeros((B, FULL, FULL), dtype=np.float32)
    xp[:, PH : PH + H, PW : PW + W] = x[:, 0]
    xs = np.ascontiguousarray(
        xp[:, 0:512, :].reshape(B, G, 128, FULL)
    ).astype(BF16)
    xtail = np.ascontiguousarray(xp[:, 512:528, :]).astype(BF16)

    # weight[(8g+j)*32+nc, (16h+u)*32+16i+v] -> wp[16j+u, g, h, i, nc, v]
    wr = weight.reshape(G, 8, 32, 2, 16, 2, 16)          # (g, j, nc, h, u, i, v)
    wpk = wr.transpose(1, 4, 0, 3, 5, 2, 6)              # (j, u, g, h, i, nc, v)
    wpk = np.ascontiguousarray(wpk.reshape(128, G, 2, 2, 32, 16)).astype(BF16)

    # selector matrices: L[16j+u, g, 8g+j] = 1
    lmat = np.zeros((128, G, 32), dtype=np.float32)
    for g in range(G):
        for j in range(8):
            lmat[16 * j : 16 * j + 16, g, 8 * g + j] = 1.0
    lm = lmat.astype(BF16)

    bpk = np.ascontiguousarray(np.tile(bias.reshape(32, 32), (4, 1)))  # [128, 32]

    in_maps = []
    for k in range(NCORES):
        in_maps.append(
            {
                "xs": np.ascontiguousarray(xs[k * BL : (k + 1) * BL]),
                "xt": np.ascontiguousarray(xtail[k * BL : (k + 1) * BL]),
                "wp": wpk,
                "lm": lm,
                "bp": bpk,
            }
        )
    return in_maps


def kernel(x, weight, bias):
    global LAST_RESULTS
    from concourse.bass_utils import run_bass_kernel_spmd

    if "nc" not in _CACHE:
        _CACHE["nc"] = _build_program()
    nc = _CACHE["nc"]

    in_maps = _prep_inputs(x, weight, bias)
    res = run_bass_kernel_spmd(
        nc, in_maps, core_ids=list(range(NCORES)), trace=TRACE
    )
    LAST_RESULTS = res
    outs = [r["out"].reshape(BL, NKH, NKW) for r in res.results]
    return np.concatenate(outs, axis=0).astype(np.float32)


# revision 17
# speedup vs baseline: 1.5372x; 1.5372x over previous
"""BlockSparseLocallyConnected forward on 8 Trainium2 NeuronCores.

Data-parallel over batch: 8 images per core, weights replicated.

out[b, nr, nc] = sum_{dr,dc} xpad[b, 16*nr+dr, 16*nc+dc] * w[(nr,nc), dr*32+dc] + bias

Decomposition: dr = 16*h + u, dc = 16*i + v (h,i in {0,1}; u,v in [0,16)),
nr = 8*g + j (g in [0,4), j in [0,8)).  Patch row = 128*g + 16*(j+h) + u.
With two row-shifted copies of the padded image (shift 0 / 16 rows), SBUF
partition p = 16*j + u holds exactly the rows needed, for both h values.
Columns 16*(nc+i)+v are free-dim strides (overlapping AP reads).

Per (b, g): DVE tensor_mul (bf16) -> product [128, (h,nc,i,v)=2048].
PE matmul with 0/1 selector lhsT L_g[16j+u, 8g+j] reduces u over partitions
and accumulates (g, h) into PSUM [128, (nc,i,v)], 4 batches per PSUM tile
(col-tile offsets 0/32/64/96).  DVE tensor_reduce(axis=XY) folds (i,v),
then bias add.  All layout shuffles/casts are host-side numpy so every DMA
is a contiguous 1:1 copy.
"""

import os
import sys

sys.path.insert(0, "/opt/trn_rl_repo")

import numpy as np
import ml_dtypes

# ---- problem constants (hardcoded; kernel.py must be self-contained) ----
B = 64            # batch
H = W = 512
PH = PW = 8
FULL = 528        # padded H/W
NKH = NKW = 32    # window grid
NCORES = 8
BL = B // NCORES  # batches per core = 8
G = 4             # window-row groups of 8 (nr = 8g + j)
WAVES = BL // 4   # psum waves per core = 2

BF16 = ml_dtypes.bfloat16

_CACHE = {}

TRACE = False          # test.py sets True to get exec_time_ns
LAST_RESULTS = None    # BassKernelResults of last run (for test.py)


def _build_program():
    import concourse.bass as bass
    import concourse.bacc as bacc
    import concourse.tile as tile
    from concourse import mybir

    dt_c = mybir.dt.bfloat16
    f32 = mybir.dt.float32

    # Bacc (not plain Bass): its compile() runs generate_event_semaphores,
    # which splits multi-wait instructions (TRN2 allows 1 wait/instruction).
    nc = bacc.Bacc(
        "TRN2", target_bir_lowering=False, debug=False, num_devices=NCORES
    )
    xs = nc.dram_tensor("xs", [BL, G, 128, 2, FULL], dt_c, kind="ExternalInput")
    wp = nc.dram_tensor("wp", [128, G, 2, 2, 32, 16], dt_c, kind="ExternalInput")
    lm = nc.dram_tensor("lm", [128, G, 32], dt_c, kind="ExternalInput")
    bp = nc.dram_tensor("bp", [128, 32], f32, kind="ExternalInput")
    out_d = nc.dram_tensor("out", [WAVES, 128, 32], f32, kind="ExternalOutput")

    with tile.TileContext(nc) as tc:
        with (
            tc.tile_pool(name="xpool", bufs=BL * G) as xpool,
            tc.tile_pool(name="cst", bufs=1) as cst,
            tc.tile_pool(name="ppool", bufs=6) as ppool,
            tc.tile_pool(name="psum", bufs=2, space="PSUM") as psum,
            tc.tile_pool(name="opool", bufs=4) as opool,
        ):
            # ONE ring (SP), strict FIFO, interleaved in exact consumption
            # order.  Head-latency fixes vs v1: the first product's prefix is
            # only x(b0,g0,s0)+w(g0,s0) (0.39MB, not 0.83MB) because both are
            # s-split, and the remaining W chunks are deferred until just
            # before their g's deadline instead of heading the stream.
            l_sb = cst.tile([128, G, 32], dt_c)
            w_all = cst.tile([128, G, 2, 2, 32, 16], dt_c)
            b_sb = cst.tile([128, 32], f32)
            x_sb = [[None] * G for _ in range(BL)]
            for b in range(BL):
                for g in range(G):
                    x_sb[b][g] = xpool.tile(
                        [128, 2, FULL], dt_c, tag="xb", name=f"xb_{b}_{g}"
                    )

            NSPLIT = 3  # first tiles of g0 s-split for a fast DVE ramp
            for s in range(2):
                nc.sync.dma_start(out=x_sb[0][0][:, s], in_=xs[0, 0, :, s])
                nc.sync.dma_start(out=w_all[:, 0, s], in_=wp[:, 0, s])
            nc.sync.dma_start(out=l_sb[:], in_=lm[:])
            for b in range(1, NSPLIT):
                for s in range(2):
                    nc.sync.dma_start(out=x_sb[b][0][:, s], in_=xs[b, 0, :, s])
            for b in range(NSPLIT, BL):
                nc.sync.dma_start(out=x_sb[b][0][:], in_=xs[b, 0])
            nc.sync.dma_start(out=w_all[:, 1], in_=wp[:, 1])
            for g in range(1, G):
                for b in range(BL):
                    nc.sync.dma_start(out=x_sb[b][g][:], in_=xs[b, g])
                    if b == 1 and g < G - 1:
                        nc.sync.dma_start(out=w_all[:, g + 1], in_=wp[:, g + 1])
            nc.sync.dma_start(out=b_sb[:], in_=bp[:])

            # PE warmup during the DMA ramp: HAM downclocks the WHOLE core
            # (PE, DVE-adjacent sequencers, and crucially the DMA fabric:
            # ~0.25 vs 0.36 MB/us) until ~3.4us of sustained PE activity.
            # vector.memset is the fastest path to a ready warm tile (DVE
            # is idle until the first product at ~11.4us anyway).
            warm = cst.tile([128, 512], dt_c)
            nc.vector.memset(warm[:], 1.0)
            wpsum = psum.tile([128, 512], f32, tag="warm")
            for _ in range(12):
                nc.tensor.matmul(wpsum[:], warm[:, 0:128], warm[:],
                                 start=True, stop=True)

            ps_tiles = [
                psum.tile([128, 32, 16], f32, tag="acc", name=f"acc{w}")
                for w in range(WAVES)
            ]

            def emit_wave_out(wv):
                # Both wave retires on DVE (GpSimd's TT path contends
                # brutally with concurrent DVE work and is slow even
                # warmed).  Wave 0 is emitted mid-stream right after the
                # (g3, b4) product, so its reduce runs as soon as its stop
                # matmul lands and its output DMA hides under the last
                # products; wave 1 is the natural tail.
                tmp = opool.tile([128, 32], f32, tag=f"tmp{wv}")
                nc.vector.tensor_reduce(
                    tmp[:], ps_tiles[wv][:],
                    axis=mybir.AxisListType.X, op=mybir.AluOpType.add,
                )
                ow = opool.tile([128, 32], f32, tag=f"ow{wv}")
                nc.vector.tensor_add(ow[:], tmp[:], b_sb[:])
                nc.scalar.dma_start(out=out_d[wv], in_=ow[:])

            for g in range(G):
                # one product per (b, g): free = (shift, i, nc*16+v); the
                # single L_g load amortizes over its 4 matmuls.  The first
                # NSPLIT products of g0 are s-split so DVE starts on the
                # half-tile that lands first.
                for b in range(BL):
                    wv, c = divmod(b, 4)
                    psum_t = ps_tiles[wv]
                    base = x_sb[b][g][:]
                    prod = ppool.tile([128, 2, 2, 512], dt_c, tag="prod")
                    s_halves = 2 if (g == 0 and b < NSPLIT) else 1
                    for sh in range(s_halves):
                        s_dim = [] if s_halves == 2 else [[FULL, 2]]
                        xview = bass.AP(
                            tensor=base.tensor,
                            offset=base.offset + sh * FULL,
                            ap=[
                                list(base.ap[0]),   # partition
                                *s_dim,             # shift (absent if split)
                                [16, 2],            # i (col offset 16i)
                                [1, 512],           # nc*16+v contiguous
                            ],
                        )
                        ws_dim = [] if s_halves == 2 else [[1024, 2]]
                        wview = bass.AP(
                            tensor=w_all.tensor,
                            offset=w_all.offset + g * 2048 + sh * 1024,
                            ap=[
                                list(w_all.ap[0]),  # partition
                                *ws_dim,            # h (=shift)
                                [512, 2],           # i
                                [1, 512],           # nc*16+v
                            ],
                        )
                        ps_dim = [] if s_halves == 2 else [[1024, 2]]
                        pview = bass.AP(
                            tensor=prod.tensor,
                            offset=prod.offset + sh * 1024,
                            ap=[
                                list(prod.ap[0]),
                                *ps_dim,
                                [512, 2],
                                [1, 512],
                            ],
                        )
                        nc.vector.tensor_mul(pview, xview, wview)
                    for s in range(2):
                        for i in range(2):
                            nc.tensor.matmul(
                                psum_t[32 * c : 32 * c + 32, :, :],
                                l_sb[:, g, :],
                                prod[:, s, i],
                                start=(g == 0 and i == 0 and s == 0),
                                stop=(g == G - 1 and i == 1 and s == 1),
                                tile_position=(0, 32 * c),
                            )
                    if g == G - 1 and b == 4:
                        emit_wave_out(0)
            emit_wave_out(1)
    nc.compile()
    return nc


def _prep_inputs(x, weight, bias):
    """Host-side packing: pad, row-shift duplicate, (j,u)-major weight shuffle,
    bf16 cast.  Returns per-core in_maps."""
    x = np.asarray(x, dtype=np.float32)
    weight = np.asarray(weight, dtype=np.float32)
    bias = np.asarray(bias, dtype=np.float32)

    xp = np.zeros((B, FULL, FULL), dtype=np.float32)
    xp[:, PH : PH + H, PW : PW + W] = x[:, 0]
    a = xp[:, 0:512, :].reshape(B, G, 128, FULL)
    bshift = xp[:, 16:528, :].reshape(B, G, 128, FULL)
    # (B, 2, G, 128, FULL) -> (B, G, 128, 2, FULL): per-(b,g) slice is a
    # fully contiguous [128, 2*FULL] block (one descriptor per partition)
    xs = np.stack([a, bshift], axis=1).transpose(0, 2, 3, 1, 4)
    xs = np.ascontiguousarray(xs).astype(BF16)

    # weight[(8g+j)*32+nc, (16h+u)*32+16i+v] -> wp[16j+u, g, h, i, nc, v]
    wr = weight.reshape(G, 8, 32, 2, 16, 2, 16)          # (g, j, nc, h, u, i, v)
    wp = wr.transpose(1, 4, 0, 3, 5, 2, 6)               # (j, u, g, h, i, nc, v)
    wp = np.ascontiguousarray(wp.reshape(128, G, 2, 2, 32, 16)).astype(BF16)

    # selector matrices: L[16j+u, g, 8g+j] = 1
    lmat = np.zeros((128, G, 32), dtype=np.float32)
    jj = np.arange(8)
    for g in range(G):
        for j in range(8):
            lmat[16 * j : 16 * j + 16, g, 8 * g + j] = 1.0
    lm = lmat.astype(BF16)

    bpk = np.ascontiguousarray(np.tile(bias.reshape(32, 32), (4, 1)))  # [128, 32]

    in_maps = []
    for k in range(NCORES):
        in_maps.append(
            {
                "xs": np.ascontiguousarray(xs[k * BL : (k + 1) * BL]),
                "wp": wp,
                "lm": lm,
                "bp": bpk,
            }
        )
    return in_maps


def kernel(x, weight, bias):
    global LAST_RESULTS
    from concourse.bass_utils import run_bass_kernel_spmd

    if "nc" not in _CACHE:
        _CACHE["nc"] = _build_program()
    nc = _CACHE["nc"]

    in_maps = _prep_inputs(x, weight, bias)
    res = run_bass_kernel_spmd(
        nc, in_maps, core_ids=list(range(NCORES)), trace=TRACE
    )
    LAST_RESULTS = res
    outs = [r["out"].reshape(BL, NKH, NKW) for r in res.results]
    return np.concatenate(outs, axis=0).astype(np.float32)

